# revision 36
# baseline (speedup 1.0000x reference)
"""Trainium2 Bass kernel for the sparse_attention nn module.

Sharding: 8 cores = 4 batches x 2 halves of the L=5120 attention rows.
Per core:
  - tiny projections (LSTM gates, K/V) run as partition-packed block-diagonal
    matmuls ([t-blocks x channels] on partitions, n=256 free), with
    per-(t,channel) biases folded into ACT bias-copy instructions
  - V(k,d) tiles via PE transposes (no DMA); K^T via a DRAM round-trip with
    the read on a HWDGE queue; X^T likewise
  - 2048->32 conv from a rearranged metadata DMA split across 4 queues
  - bilinear grid-sample one-hot outer products split DVE/GPSIMD, copies
    split DVE/ACT; window-1 grid weights + Q path run in window-0's shadow
  - Q computed directly from [XT; lcT] with host-fused fc@vf weights
  - attention: scores^T = K @ Q^T in two 1280-query windows per k-tile,
    sigmoid on ACT (the binding engine; 80 x [128,1280] tiles back-to-back),
    out-projection as probs-as-weights matmuls accumulating [128l, 8]
    column-groups in PSUM over all 40 k-tiles
  - epilogue: threshold+mask on DVE, one PE transpose per window, fco as a
    block-diagonal matmul, bias on DVE, strided DMA out
"""
import sys

sys.path.insert(0, "/opt/trn_rl_repo")

import numpy as np

import concourse.bacc as bacc
import concourse.tile as tile
from concourse import mybir
from concourse.bass_utils import run_bass_kernel_spmd
from concourse.masks import make_identity

F32 = mybir.dt.float32
BF16 = mybir.dt.bfloat16
ALU = mybir.AluOpType
ACTF = mybir.ActivationFunctionType
F32R = mybir.dt.float32r

B, T, N = 4, 20, 256
L = T * N            # 5120
HL = L // 2          # 2560 rows per core
HT = T // 2          # 10 t-steps per core
CMAP, CC = 2048, 32
NK = L // 128        # 40 k-tiles
FW = 1280            # query-window width
NW = HL // FW        # 2 windows
NCH = FW // 128      # 10 out column-chunks per window

# csth (f32r weight blob) column layout
C_KW, C_VW, C_GWI, C_GWO, C_GG = 0, 80, 160, 200, 240
C_QX, C_QLC = 280, 288
CSTH_W = 320
# bia (f32 per-partition bias blob) column layout
B_KB0, B_KB1, B_VB0, B_VB1, B_GBI, B_GBO, B_GBG, B_QB, B_FCOB, B_CMPB = (
    0, 1, 2, 3, 4, 5, 6, 7, 8, 9)
BIA_W = 10

_nc_cache = None
DEBUG = False


def _build():
    nc = bacc.Bacc()
    dt_in = {
        "xpk": ([64, 256], F32R),     # rows 2t+c (t0-9), 32+2t'+c (t10-19)
        "xg": ([20, 256], F32R),      # core's half t-local: rows 2tl+c
        "xpm": ([128, 20, 2], F32),   # point-major half coords
        "csth": ([64, CSTH_W], F32R),
        "bia": ([80, BIA_W], F32),
        "fcow": ([80, 20], F32R),     # block-diagonal fco_w^T per 8-row block
        "mdh": ([128, 16, 256], F32R),
        "cwh": ([128, 16, CC], F32R),
    }
    d = {k: nc.dram_tensor(k, sh, dt, kind="ExternalInput")
         for k, (sh, dt) in dt_in.items()}
    y_out = nc.dram_tensor("y", [2, HL], F32, kind="ExternalOutput")
    dr_X = nc.dram_tensor("dr_X", [40, 256], F32R, kind="Internal")
    dr_Kp = nc.dram_tensor("dr_Kp", [80, 2, 256], F32R, kind="Internal")

    with tile.TileContext(nc) as tc:
        with tc.tile_pool(name="main", bufs=1) as pool, \
             tc.tile_pool(name="work", bufs=3) as work, \
             tc.tile_pool(name="work2", bufs=2) as work2, \
             tc.tile_pool(name="ps", bufs=2, space="PSUM") as psp, \
             tc.tile_pool(name="po", bufs=1, space="PSUM") as pop, \
             tc.tile_pool(name="pe", bufs=1, space="PSUM") as pep:

            # ---- input DMAs spread across the three DMA queues ----
            sb_xpm = pool.tile([128, 20, 2], F32)
            nc.sync.dma_start(sb_xpm, d["xpm"].ap())
            sb_xg = pool.tile([20, 256], F32R)
            nc.sync.dma_start(sb_xg, d["xg"].ap())
            sb_csth = pool.tile([64, CSTH_W], F32R)
            nc.sync.dma_start(sb_csth, d["csth"].ap())
            sb_xpk = pool.tile([64, 256], F32R)
            nc.sync.dma_start(sb_xpk, d["xpk"].ap())
            sb_cwh = pool.tile([128, 16, CC], F32R)
            nc.sync.dma_start(sb_cwh, d["cwh"].ap())
            sb_mdh = pool.tile([128, 16, 256], F32R)
            nc.sync.dma_start(sb_mdh[:, 0:4, :], d["mdh"].ap()[:, 0:4, :])
            nc.sync.dma_start(sb_mdh[:, 4:8, :], d["mdh"].ap()[:, 4:8, :])
            sb_bia = pool.tile([80, BIA_W], F32)
            nc.gpsimd.dma_start(sb_bia, d["bia"].ap())
            sb_fcow = pool.tile([80, 20], F32R)
            nc.gpsimd.dma_start(sb_fcow, d["fcow"].ap())
            nc.gpsimd.dma_start(sb_mdh[:, 8:12, :], d["mdh"].ap()[:, 8:12, :])

            def W(c0, c1, r0=0, r1=20):
                return sb_csth[r0:r1, c0:c1]

            def bias(col, rows):
                return sb_bia[0:rows, col:col + 1]

            # dummy sigmoid+tanh pull a combined ACT table load to t=0
            warm = pool.tile([1, 1], F32)
            nc.vector.memset(warm, 0.0)
            warm2 = pool.tile([1, 1], F32)
            nc.scalar.activation(warm2, warm, ACTF.Sigmoid)
            warm3 = pool.tile([1, 1], F32)
            nc.scalar.activation(warm3, warm, ACTF.Tanh)
            # last metadata quarter on the ACT queue
            nc.scalar.dma_start(sb_mdh[:, 12:16, :], d["mdh"].ap()[:, 12:16, :])
            identf = pool.tile([128, 128], F32)
            make_identity(nc, identf)
            identb = pool.tile([128, 128], BF16)
            make_identity(nc, identb)
            iota16 = pool.tile([128, 16], F32)
            nc.gpsimd.iota(iota16, [[1, 16]], base=0, channel_multiplier=0,
                           allow_small_or_imprecise_dtypes=True)

            # ---- persistent SBUF results ----
            sb_KT = pool.tile([8, L], F32R)
            sb_QT = pool.tile([8, HL], F32R)
            sb_Vkd = pool.tile([128, 2, 2, HT, 8], BF16)  # (a, h, tl, d)
            sb_WgT = [pool.tile([128, HL], F32R, name=f"wgT{h}") for h in range(2)]
            sb_lc = pool.tile([CC, HL], F32R)
            sb_XT = pool.tile([4, HL], F32R)
            sb_cm = pool.tile([CC, 256], F32)
            sb_cmT = pool.tile([128, 2, CC], F32R)
            sb_Kp = pool.tile([80, 2, 256], F32R)
            sb_Vp = pool.tile([80, 2, 256], BF16)

            # ---- LSTM gates (packed [4*tl + ch, n]) ----
            ps_i = psp.tile([40, 256], F32, tag="s", name="ps_i")
            nc.tensor.matmul(ps_i, lhsT=W(C_GWI, C_GWO), rhs=sb_xg)
            Xi = work2.tile([40, 256], BF16, tag="Xi")
            nc.scalar.activation(Xi, ps_i, ACTF.Sigmoid, bias=bias(B_GBI, 40))
            ps_o2 = psp.tile([40, 256], F32, tag="s", name="ps_o2")
            nc.tensor.matmul(ps_o2, lhsT=W(C_GWO, C_GG), rhs=sb_xg)
            Xo = work2.tile([40, 256], BF16, tag="Xo")
            nc.scalar.activation(Xo, ps_o2, ACTF.Sigmoid, bias=bias(B_GBO, 40))
            ps_g = psp.tile([40, 256], F32, tag="s", name="ps_g")
            nc.tensor.matmul(ps_g, lhsT=W(C_GG, C_GG + 40), rhs=sb_xg)
            Xg = work2.tile([40, 256], BF16, tag="Xg")
            nc.scalar.activation(Xg, ps_g, ACTF.Tanh, bias=bias(B_GBG, 40))
            Xc = work2.tile([40, 256], BF16, tag="Xc")
            nc.vector.tensor_tensor(Xc, Xi, Xg, ALU.mult)
            # ---- K/V projections (packed) ----
            for a in range(2):
                r0 = 32 * a
                rx = sb_xpk[r0:r0 + 20, :]
                ps_kp = psp.tile([80, 256], F32, tag="s", name="ps_kp")
                nc.tensor.matmul(ps_kp, lhsT=W(C_KW, C_VW, r0, r0 + 20), rhs=rx)
                nc.scalar.activation(sb_Kp[:, a, :], ps_kp, ACTF.Identity,
                                     bias=bias(B_KB0 + a, 80))
                ps_vp = psp.tile([80, 256], F32, tag="s", name="ps_vp")
                nc.tensor.matmul(ps_vp, lhsT=W(C_VW, C_GWI, r0, r0 + 20), rhs=rx)
                nc.scalar.activation(sb_Vp[:, a, :], ps_vp, ACTF.Identity,
                                     bias=bias(B_VB0 + a, 80))
            nc.gpsimd.dma_start(dr_Kp.ap(), sb_Kp)

            Xc2 = work2.tile([40, 256], BF16, tag="Xc2")
            nc.scalar.activation(Xc2, Xc, ACTF.Tanh)
            sb_X = pool.tile([40, 256], F32R)
            nc.vector.tensor_tensor(sb_X, Xo, Xc2, ALU.mult)
            nc.gpsimd.dma_start(dr_X.ap(), sb_X)

            # V(k,d) tiles via PE transpose: Vp[:, a, 128h:...]^T = [n, (tl d)]
            for a in range(2):
                for h in range(2):
                    ps_v = psp.tile([128, 80], BF16, tag="s", name="ps_v")
                    nc.tensor.transpose(ps_v, sb_Vp[:, a, 128 * h:128 * (h + 1)],
                                        identb[0:80, 0:80])
                    nc.vector.tensor_copy(
                        sb_Vkd[:, a, h, :, :].rearrange("p t c -> p (t c)"),
                        ps_v)

            # K^T [8, L] via DRAM round-trip; reads on the SP (HWDGE) queue
            for a in range(2):
                nc.sync.dma_start(
                    sb_KT[:, a * HL:(a + 1) * HL].rearrange(
                        "c (t n) -> c t n", t=HT),
                    dr_Kp.ap()[:, a, :].rearrange("(t c) n -> c t n", c=8))
            nc.sync.dma_start(
                sb_XT.rearrange("c (t n) -> c t n", t=HT),
                dr_X.ap().rearrange("(t c) n -> c t n", c=4))

            # ---- compressed feature map cm then cmT ----
            ps_cm = psp.tile([CC, 256], F32, tag="s", name="ps_cm")
            for k in range(16):
                nc.tensor.matmul(ps_cm, lhsT=sb_cwh[:, k, :],
                                 rhs=sb_mdh[:, k, :],
                                 start=(k == 0), stop=(k == 15))
            nc.scalar.activation(sb_cm, ps_cm, ACTF.Identity, bias=bias(B_CMPB, CC))
            for h in range(2):
                ps_ct = psp.tile([128, CC], F32, tag="s", name="ps_ct")
                nc.tensor.transpose(ps_ct,
                                    sb_cm[:, h * 128:(h + 1) * 128],
                                    identf[0:CC, 0:CC])
                nc.vector.tensor_copy(sb_cmT[:, h, :], ps_ct)

            # ---- grid-sample weights (per-point scalars, big-tile DVE) ----
            ixy = pool.tile([128, 20, 2], F32)
            nc.vector.tensor_scalar(ixy, sb_xpm, 1.0 / 32.0, 0.5, ALU.mult, ALU.add)
            ti = pool.tile([128, 20, 2], mybir.dt.int32)
            nc.vector.tensor_copy(ti, ixy)
            tf = pool.tile([128, 20, 2], F32)
            nc.vector.tensor_copy(tf, ti)
            gt = pool.tile([128, 20, 2], F32)
            nc.vector.tensor_tensor(gt, tf, ixy, ALU.is_gt)
            x0f = pool.tile([128, 20, 2], F32)   # = floor coord + 1, in [0,16]
            nc.vector.tensor_tensor(x0f, tf, gt, ALU.subtract)
            fr = pool.tile([128, 20, 2], F32)
            nc.vector.tensor_tensor(fr, ixy, x0f, ALU.subtract)
            w0 = pool.tile([128, 20, 2], F32)
            nc.vector.tensor_scalar(w0, fr, -1.0, 1.0, ALU.mult, ALU.add)
            v0 = pool.tile([128, 20, 2], F32)
            nc.vector.tensor_scalar(v0, x0f, 0.5, None, ALU.is_ge)
            v1 = pool.tile([128, 20, 2], F32)
            nc.vector.tensor_scalar(v1, x0f, 15.5, None, ALU.is_le)
            w0e = pool.tile([128, 20, 2], F32)
            nc.vector.tensor_tensor(w0e, w0, v0, ALU.mult)
            w1e = pool.tile([128, 20, 2], F32)
            nc.vector.tensor_tensor(w1e, fr, v1, ALU.mult)
            x0c = pool.tile([128, 20, 2], F32)
            nc.vector.tensor_scalar(x0c, x0f, -1.0, 0.0, ALU.add, ALU.max)
            x1c = pool.tile([128, 20, 2], F32)
            nc.vector.tensor_scalar(x1c, x0f, 15.0, None, ALU.min)

            # one-hot corner weights Ox (DVE lane), Oy (GPSIMD lane)
            ohs = []
            oh_tmp = []
            for a in range(2):
                o_t = pool.tile([128, 20, 16], F32, name=f"oh{a}")
                tmp = pool.tile([128, 20, 16], F32, name=f"ohtmp{a}")
                ohs.append(o_t)
                oh_tmp.append(tmp)

            def emit_oh(c0, c1):
                n = c1 - c0
                sh = (128, n, 16)
                for a, eng in ((0, nc.vector), (1, nc.vector)):
                    o_t = ohs[a][:, c0:c1, :]
                    tmp = oh_tmp[a][:, c0:c1, :]
                    eng.tensor_tensor(
                        o_t, iota16[:, None, :].to_broadcast(sh),
                        x0c[:, c0:c1, a:a + 1].to_broadcast(sh), ALU.is_equal)
                    eng.tensor_tensor(
                        o_t, o_t, w0e[:, c0:c1, a:a + 1].to_broadcast(sh),
                        ALU.mult)
                    eng.tensor_tensor(
                        tmp, iota16[:, None, :].to_broadcast(sh),
                        x1c[:, c0:c1, a:a + 1].to_broadcast(sh), ALU.is_equal)
                    eng.tensor_tensor(
                        tmp, tmp, w1e[:, c0:c1, a:a + 1].to_broadcast(sh),
                        ALU.mult)
                    eng.tensor_tensor(o_t, o_t, tmp, ALU.add)
            emit_oh(0, 10)
            Ox, Oy = ohs

            # Wg[p, (py,px)] = Oy*Ox outer products; PE-transpose pairs of
            # 128-chunks into one PSUM tile, two copies out per pair.
            def emit_pair(pc, outer_eng, copy_engs, pspool, ptag="s"):
                wgs = []
                for j in range(2):
                    c = 2 * pc + j
                    wg = work.tile([128, 16, 16], F32, tag=f"wg{j}",
                                   name=f"wg{j}")
                    outer_eng.tensor_tensor(
                        wg, Oy[:, c, :, None].to_broadcast((128, 16, 16)),
                        Ox[:, c, None, :].to_broadcast((128, 16, 16)),
                        ALU.mult)
                    wgs.append(wg.rearrange("p a b -> p (a b)"))
                for h in range(2):
                    ps_p = pspool.tile([128, 256], F32, tag=ptag, name="ps_p")
                    for j in range(2):
                        nc.tensor.transpose(
                            ps_p[:, j * 128:(j + 1) * 128],
                            wgs[j][:, h * 128:(h + 1) * 128],
                            identf)
                    eng = copy_engs[h]
                    if eng is nc.scalar:
                        nc.scalar.activation(
                            sb_WgT[h][:, pc * 256:(pc + 1) * 256], ps_p,
                            ACTF.Identity)
                    else:
                        eng.tensor_copy(
                            sb_WgT[h][:, pc * 256:(pc + 1) * 256], ps_p)
            # window-0 pairs up front: outers alternate DVE/GPSIMD, copies
            # split DVE/ACT
            for pc in range(5):
                emit_pair(pc, nc.vector, (nc.vector, nc.scalar), psp)

            # ---- Q path: lcT then Q = Wqx@XT + Wqlc@lcT ----
            def emit_lcq(w, cp_eng, pspool, cw, ptag="s"):
                for j0 in range(0, FW, cw):
                    cwe = min(cw, FW - j0)
                    qsl = slice(w * FW + j0, w * FW + j0 + cwe)
                    ps_lc = pspool.tile([CC, cwe], F32, tag=ptag, name="ps_lc")
                    for j in range(0, cwe, 512):
                        je = min(j + 512, cwe)
                        for h in range(2):
                            nc.tensor.matmul(
                                ps_lc[:, j:je], lhsT=sb_cmT[:, h, :],
                                rhs=sb_WgT[h][:, qsl][:, j:je],
                                start=(h == 0), stop=(h == 1))
                    if cp_eng is nc.scalar:
                        nc.scalar.activation(sb_lc[:, qsl], ps_lc, ACTF.Identity)
                    else:
                        cp_eng.tensor_copy(sb_lc[:, qsl], ps_lc)
                    ps_q = pspool.tile([8, cwe], F32, tag=ptag, name="ps_q")
                    for j in range(0, cwe, 512):
                        je = min(j + 512, cwe)
                        nc.tensor.matmul(ps_q[:, j:je],
                                         lhsT=W(C_QX, C_QX + 8, 0, 4),
                                         rhs=sb_XT[:, qsl][:, j:je],
                                         start=True, stop=False)
                        nc.tensor.matmul(ps_q[:, j:je],
                                         lhsT=W(C_QLC, C_QLC + 8, 0, CC),
                                         rhs=sb_lc[:, qsl][:, j:je],
                                         start=False, stop=True)
                    cp2 = nc.vector if cp_eng is nc.scalar else cp_eng
                    cp2.tensor_scalar(sb_QT[:, qsl], ps_q, bias(B_QB, 8),
                                      None, ALU.add)
            with tc.high_priority():
                emit_lcq(0, nc.scalar, psp, 512)

            # ---- attention: ACT-bound sigmoid pipeline ----
            def vtile(ki):
                return sb_Vkd[:, ki // 20, ki % 2, (ki % 20) // 2, :]

            def emit_scores(w, ki, ps_s):
                for j in range(0, FW, 512):
                    je = min(j + 512, FW)
                    nc.tensor.matmul(ps_s[:, j:je],
                                     lhsT=sb_KT[:, ki * 128:(ki + 1) * 128],
                                     rhs=sb_QT[:, w * FW + j:w * FW + je],
                                     start=True, stop=True)

            def emit_outs(w, ki, po, probs):
                # start only once per bank: start_tensor_calc marks the whole
                # 2KB zero region pending; sibling chunks are zeroed lazily by
                # their own first write
                for c in range(NCH):
                    nc.tensor.matmul(po[:, 8 * c:8 * c + 8],
                                     lhsT=probs[:, c * 128:(c + 1) * 128],
                                     rhs=vtile(ki),
                                     start=(ki == 0 and c == 0),
                                     stop=(ki == NK - 1),
                                     skip_group_check=True)

            def emit_epilogue(w, po):
                qsl = slice(w * FW, (w + 1) * FW)
                o_s = work2.tile([128, 8 * NCH], F32, tag="os", name="o_s")
                nc.vector.tensor_copy(o_s, po)
                msk = work2.tile([128, 8 * NCH], F32, tag="msk", name="msk")
                nc.vector.tensor_scalar(msk, o_s, 0.5, None, ALU.is_gt)
                o_m = work2.tile([128, 8 * NCH], F32, tag="om", name="o_m")
                nc.vector.tensor_tensor(o_m, o_s, msk, ALU.mult)
                ps_t = pep.tile([8 * NCH, 128], F32, tag="t", name="ps_t")
                nc.tensor.transpose(ps_t, o_m, identf)
                sb_t = work2.tile([8 * NCH, 128], F32R, tag="st", name="sb_t")
                nc.vector.tensor_copy(sb_t, ps_t)
                ps_y = pep.tile([2 * NCH, 128], F32, tag="t", name="ps_y")
                nc.tensor.matmul(ps_y, lhsT=sb_fcow, rhs=sb_t)
                sb_yw = work2.tile([2 * NCH, 128], F32, tag="yw", name="sb_yw")
                nc.vector.tensor_scalar(sb_yw, ps_y, bias(B_FCOB, 2 * NCH),
                                        None, ALU.add)
                for d2 in range(2):
                    nc.sync.dma_start(
                        y_out.ap()[d2:d2 + 1, qsl].rearrange(
                            "d (c j) -> (d c) j", c=NCH),
                        sb_yw[NCH * d2:NCH * d2 + NCH, :])

            po_all = pop.tile([128, NW * 8 * NCH], F32, tag="o", name="po_all")
            po_t = [po_all[:, 8 * NCH * w:8 * NCH * (w + 1)] for w in range(NW)]
            ps_prev = None
            for w in range(NW):
                for ki in range(NK):
                    if w == 0 and ki == 0:
                        with tc.high_priority():
                            ps_s = psp.tile([128, FW], F32, tag="s",
                                            name="ps_s")
                            emit_scores(0, 0, ps_s)
                        ps_prev = ps_s
                        emit_oh(10, 20)
                    probs = work.tile([128, FW], BF16, tag="p", name="probs")
                    nc.scalar.activation(probs, ps_prev, ACTF.Sigmoid)
                    # next scores first (PE in-order: avoid stalling on sigmoid)
                    nw_, nk = (w, ki + 1) if ki + 1 < NK else (w + 1, 0)
                    if nw_ < NW:
                        ps_s = psp.tile([128, FW], F32, tag="s", name="ps_s")
                        emit_scores(nw_, nk, ps_s)
                        ps_prev = ps_s
                    emit_outs(w, ki, po_t[w], probs)
                    # window-1 grid weights + Q path in window-0's shadow,
                    # using the spare PSUM bank (pep)
                    if w == 0 and ki >= 2 and ki < 17 and (ki - 2) % 3 == 0:
                        emit_pair(5 + (ki - 2) // 3, nc.vector,
                                  (nc.vector, nc.vector), pep, ptag="t")
                    if w == 0 and ki == 20:
                        emit_lcq(1, nc.vector, pep, 512, ptag="t")
                emit_epilogue(w, po_t[w])

            if DEBUG:
                for nm, t in (("dbg_KT", sb_KT), ("dbg_QT", sb_QT),
                              ("dbg_lc", sb_lc), ("dbg_X", sb_X),
                              ("dbg_XT", sb_XT),
                              ("dbg_cm", sb_cm), ("dbg_Vkd", sb_Vkd)):
                    dt_o = nc.dram_tensor(nm, list(t.shape), t.dtype,
                                          kind="ExternalOutput")
                    nc.sync.dma_start(dt_o.ap(), t)

    nc.compile()
    return nc


def _prep_inputs(x, metadata, w_ih, b_ih, b_hh, comp_w, comp_b, vf_w, vf_b,
                 fc_w, fc_b, fc2_w, fc2_b, fc3_w, fc3_b, fco_w, fco_b):
    f = np.float32
    pos = np.arange(T, dtype=f)
    pe = np.stack([np.sin(pos), np.cos(pos)], axis=-1).astype(f)  # (T,2)
    w_ih = np.asarray(w_ih, f)
    bb = np.asarray(b_ih, f) + np.asarray(b_hh, f)
    w_i, w_g, w_o = w_ih[0:4], w_ih[8:12], w_ih[12:16]
    gb_i = (pe @ w_i.T + bb[0:4]).T          # (4, T)
    gb_g = (pe @ w_g.T + bb[8:12]).T
    gb_o = (pe @ w_o.T + bb[12:16]).T
    fc2_w = np.asarray(fc2_w, f)
    fc3_w = np.asarray(fc3_w, f)
    kb = (pe @ fc2_w.T + np.asarray(fc2_b, f)).T   # (8, T)
    vb = (pe @ fc3_w.T + np.asarray(fc3_b, f)).T

    # fused Q weights: Q = fc_w @ (vf_w @ [X; lc]) + (fc_w @ vf_b + fc_b)
    vf_w = np.asarray(vf_w, f)
    fc_w = np.asarray(fc_w, f)
    wq = fc_w @ vf_w                                # (8, 36)
    qb = fc_w @ np.asarray(vf_b, f) + np.asarray(fc_b, f)   # (8,)

    # f32r weight blob
    csth = np.zeros((64, CSTH_W), f)
    for tl in range(10):
        r0, r1 = 2 * tl, 32 + 2 * tl
        cK, cV = C_KW + 8 * tl, C_VW + 8 * tl
        for c in range(2):
            csth[r0 + c, cK:cK + 8] = fc2_w[:, c]
            csth[r1 + c, cK:cK + 8] = fc2_w[:, c]
            csth[r0 + c, cV:cV + 8] = fc3_w[:, c]
            csth[r1 + c, cV:cV + 8] = fc3_w[:, c]
            csth[r0 + c, C_GWI + 4 * tl:C_GWI + 4 * tl + 4] = w_i[:, c]
            csth[r0 + c, C_GWO + 4 * tl:C_GWO + 4 * tl + 4] = w_o[:, c]
            csth[r0 + c, C_GG + 4 * tl:C_GG + 4 * tl + 4] = w_g[:, c]
    csth[0:4, C_QX:C_QX + 8] = wq[:, 0:4].T
    csth[0:CC, C_QLC:C_QLC + 8] = wq[:, 4:36].T
    csth = np.ascontiguousarray(csth)

    fco_w = np.asarray(fco_w, f)
    fcow = np.zeros((80, 20), f)
    for c in range(10):
        for d2 in range(2):
            fcow[8 * c:8 * c + 8, 10 * d2 + c] = fco_w[d2, :]
    fcow = np.ascontiguousarray(fcow)

    cwh = np.ascontiguousarray(
        np.asarray(comp_w, f).T.reshape(16, 128, CC).transpose(1, 0, 2))

    in_maps = []
    xf = np.asarray(x, f)
    mdf = np.asarray(metadata, f)
    for core in range(8):
        b_, hi = core // 2, core % 2
        xb = xf[b_]                       # (2, T, N)
        xpk = np.zeros((64, 256), f)
        for t in range(10):
            xpk[2 * t:2 * t + 2, :] = xb[:, t, :]
            xpk[32 + 2 * t:32 + 2 * t + 2, :] = xb[:, 10 + t, :]
        xg = np.zeros((20, 256), f)
        for tl in range(10):
            xg[2 * tl:2 * tl + 2, :] = xb[:, 10 * hi + tl, :]
        xh = np.ascontiguousarray(xb[:, 10 * hi:10 * hi + 10, :]).reshape(2, HL)
        xpm = np.ascontiguousarray(
            xh.reshape(2, 20, 128).transpose(2, 1, 0))     # (128, 20, 2)

        bia = np.zeros((80, BIA_W), f)
        for tl in range(10):
            bia[8 * tl:8 * tl + 8, B_KB0] = kb[:, tl]
            bia[8 * tl:8 * tl + 8, B_KB1] = kb[:, 10 + tl]
            bia[8 * tl:8 * tl + 8, B_VB0] = vb[:, tl]
            bia[8 * tl:8 * tl + 8, B_VB1] = vb[:, 10 + tl]
            t = 10 * hi + tl
            bia[4 * tl:4 * tl + 4, B_GBI] = gb_i[:, t]
            bia[4 * tl:4 * tl + 4, B_GBO] = gb_o[:, t]
            bia[4 * tl:4 * tl + 4, B_GBG] = gb_g[:, t]
        bia[0:8, B_QB] = qb
        bia[0:20, B_FCOB] = np.repeat(np.asarray(fco_b, f), 10)
        bia[0:CC, B_CMPB] = np.asarray(comp_b, f)

        mdh = np.ascontiguousarray(
            mdf[b_].reshape(CMAP, 256).reshape(16, 128, 256).transpose(1, 0, 2))

        in_maps.append(dict(
            xpk=xpk, xg=xg, xpm=xpm,
            csth=csth, bia=bia, fcow=fcow, mdh=mdh, cwh=cwh))
    return in_maps


def kernel(**inputs):
    global _nc_cache
    if _nc_cache is None:
        _nc_cache = _build()
    in_maps = _prep_inputs(**inputs)
    res = run_bass_kernel_spmd(_nc_cache, in_maps, core_ids=list(range(8)))
    out = np.zeros((B, 2, T, N), np.float32)
    for core in range(8):
        b_, hi = core // 2, core % 2
        y = np.asarray(res.results[core]["y"]).reshape(2, HT, N)
        out[b_, :, hi * HT:(hi + 1) * HT, :] = y
    return out
